# revision 14
# baseline (speedup 1.0000x reference)
"""Trainium2 Bass kernel for a tanh RNN (h_t = tanh(x_t @ W + h_{t-1} @ U + b)).

Strategy
--------
Data-parallel over batch: 64 sequences -> 8 cores x 8 sequences. W/U/b are
replicated; the recurrent state stays resident per core.

Per core, the T=2048 serial scan is reformulated as a block-Jacobi scan:
split T into NB=64 blocks of TB=32 steps. All blocks are scanned in
parallel (they become extra batch: 8 seqs x 64 blocks = 512 columns per
step), seeded with initial states from the previous sweep (zeros on sweep
0). Two sweeps suffice: the per-step Jacobian ||diag(tanh') U|| ~ 0.7
contracts an O(1) seed error to ~1e-5 across a 32-step block (verified
against the reference scan). Sweep 2 re-scans with block i seeded by
sweep-1's end state of block i-1 and emits the outputs.

Layout: the scan state is kept transposed (hT: units on partitions, batch
in the free dim) so tanh (ScalarE) reads PSUM and writes hT directly with
no per-step transpose. The 512-column batch is split into two independent
groups (s 0-3 / s 4-7) whose chains interleave, so one group's tanh
overlaps the other group's matmuls. Per step and group: 6 matmuls into
one PSUM bank — z.T[uo, col] = W[:, uo].T @ xT_t (start=True injects x@W
and clears the bank) plus four accumulating U-block matmuls — then one
ACT tanh over [128, 512]. Matmuls run as float32r (full-rate fp32; plain
fp32 is 4 cycles/row), which lands ~1e-3 scale-relative error on this
contractive recurrence.

Host-side shard glue: x is uploaded pre-transposed and step-major
(xT[:, t*512 + col]) so the contraction dim lands on partitions, each
step reads a contiguous slab, and the input DMA streams just ahead of
the scan. y is written to DRAM in the scan's native layout and
un-transposed on the host during the gather. All FLOPs stay on device;
the host only does layout.
"""

import os

import numpy as np

B_GLOB = 64
B_LOC = 8
T = 2048
F = 128
H = 256
NCORES = 8
TB = 32                    # block length (chain steps per sweep)
NB = T // TB               # 64 blocks
BATCH = B_LOC * NB         # 512 columns per scan step
NG = 2                     # independent batch groups (interleaved chains)
GW = BATCH // NG           # 256 columns per group
ROWS = B_LOC * T           # 16384 columns of xT per core

_CACHE = {}


def _build(has_bias: bool, use_f32r: bool = True):
    from contextlib import ExitStack

    import concourse.tile as tile
    from concourse import bacc, mybir

    f32 = mybir.dt.float32
    # float32r: same fp32 storage, tensor-engine matmul runs at full rate
    # (plain fp32 is 4 cycles/row) with reduced product precision; fine here
    # because the recurrence is contractive.
    mmdt = mybir.dt.float32r if use_f32r else f32

    def mm(ap):
        return ap if ap.dtype == mmdt else ap.bitcast(mmdt)

    nc = bacc.Bacc(
        "TRN2",
        target_bir_lowering=False,
        debug=False,
        enable_asserts=False,
        num_devices=NCORES,
    )

    # xT columns: t*BATCH + g*GW + s_loc*NB + blk   (step-major, contiguous)
    xT_d = nc.dram_tensor("xt", (F, TB, NG, GW), f32, kind="ExternalInput").ap()
    w_d = nc.dram_tensor("w", (F, H), f32, kind="ExternalInput").ap()
    u_d = nc.dram_tensor("u", (H, H), f32, kind="ExternalInput").ap()
    if has_bias:
        b_d = nc.dram_tensor("bvec", (H,), f32, kind="ExternalInput").ap()
    # y in scan layout: [t][g][u%128][half*GW + s_loc*NB + blk]
    y_d = nc.dram_tensor("yscr", (TB, NG, 128, 2 * GW), f32, kind="ExternalOutput").ap()

    with tile.TileContext(nc) as tc, ExitStack() as ctx:
        consts = ctx.enter_context(tc.tile_pool(name="consts", bufs=1))
        hpool = ctx.enter_context(tc.tile_pool(name="hpool", bufs=6))
        zpsum = ctx.enter_context(tc.tile_pool(name="zpsum", bufs=4, space="PSUM"))

        w_sb = consts.tile([128, H], mmdt)
        nc.sync.dma_start(out=w_sb, in_=mm(w_d))
        u_sb = consts.tile([128, 2, H], mmdt)
        nc.sync.dma_start(out=u_sb, in_=mm(u_d.rearrange("(k p) h -> p k h", p=128)))
        if has_bias:
            b_sb = consts.tile([128, 2], f32)
            nc.sync.dma_start(out=b_sb, in_=b_d.rearrange("(k p) -> p k", p=128))

        # x, pre-transposed/step-major on host: [128(f), TB, NG, GW]
        xT = consts.tile([128, TB, NG, GW], mmdt)
        # stream the load in step-order so the scan can start immediately
        NLOAD = 16
        CHT = TB // NLOAD
        for c in range(NLOAD):
            nc.sync.dma_start(
                out=xT[:, c * CHT : (c + 1) * CHT],
                in_=mm(xT_d[:, c * CHT : (c + 1) * CHT]),
            )

        tanh = mybir.ActivationFunctionType.Tanh

        zeros_sb = consts.tile([128, 2 * GW], f32)
        nc.vector.memset(zeros_sb, 0.0)

        h_prev = []
        for g in range(NG):
            hp = hpool.tile([128, 2 * GW], mmdt, tag=f"h{g}")
            nc.vector.tensor_copy(out=hp[:], in_=zeros_sb[:])
            h_prev.append(hp)

        for p in range(2):
            final = p == 1
            for t in range(TB):
                for g in range(NG):
                    xmov = xT[:, t, g, :]  # [128, GW] contiguous
                    hp = h_prev[g]
                    z = zpsum.tile([128, 2 * GW], f32, tag=f"z{g}")
                    # inject x_t @ W (start=True clears the bank)
                    nc.tensor.matmul(
                        z[:, 0:GW], lhsT=mm(w_sb[:, 0:128]), rhs=mm(xmov),
                        start=True, stop=False,
                    )
                    nc.tensor.matmul(
                        z[:, GW : 2 * GW], lhsT=mm(w_sb[:, 128:256]), rhs=mm(xmov),
                        start=False, stop=False,
                    )
                    # accumulate h_{t-1} @ U
                    nc.tensor.matmul(
                        z[:, 0:GW], lhsT=mm(u_sb[:, 0, 0:128]),
                        rhs=mm(hp[:, 0:GW]), start=False, stop=False,
                    )
                    nc.tensor.matmul(
                        z[:, 0:GW], lhsT=mm(u_sb[:, 1, 0:128]),
                        rhs=mm(hp[:, GW : 2 * GW]), start=False, stop=False,
                    )
                    nc.tensor.matmul(
                        z[:, GW : 2 * GW], lhsT=mm(u_sb[:, 0, 128:256]),
                        rhs=mm(hp[:, 0:GW]), start=False, stop=False,
                    )
                    nc.tensor.matmul(
                        z[:, GW : 2 * GW], lhsT=mm(u_sb[:, 1, 128:256]),
                        rhs=mm(hp[:, GW : 2 * GW]), start=False, stop=True,
                    )

                    h_cur = hpool.tile([128, 2 * GW], mmdt, tag=f"h{g}")
                    if has_bias:
                        nc.scalar.activation(
                            out=h_cur[:, 0:GW], in_=z[:, 0:GW],
                            func=tanh, bias=b_sb[:, 0:1],
                        )
                        nc.scalar.activation(
                            out=h_cur[:, GW : 2 * GW], in_=z[:, GW : 2 * GW],
                            func=tanh, bias=b_sb[:, 1:2],
                        )
                    else:
                        nc.scalar.activation(out=h_cur[:], in_=z[:], func=tanh)

                    if final:
                        nc.sync.dma_start(
                            out=y_d[t, g],
                            in_=h_cur[:].bitcast(f32),
                        )

                    h_prev[g] = h_cur

            if not final:
                # seed sweep 2: block i <- end state of block i-1; block 0 <- 0
                for g in range(NG):
                    ends = h_prev[g]
                    h_init = hpool.tile([128, 2 * GW], mmdt, tag=f"h{g}")
                    nc.vector.tensor_copy(out=h_init[:], in_=zeros_sb[:])
                    # cols: (half, s_loc, blk); shift blk by one within (half, s_loc)
                    ends_g = ends[:].rearrange("p (q nb) -> p q nb", nb=NB)
                    init_g = h_init[:].rearrange("p (q nb) -> p q nb", nb=NB)
                    nc.vector.tensor_copy(
                        out=init_g[:, :, 1:NB], in_=ends_g[:, :, 0 : NB - 1]
                    )
                    h_prev[g] = h_init

    nc.compile()
    return nc


def _get_program(has_bias: bool):
    # RNN_EXACT=1 switches the matmuls to exact fp32 (4 cycles/row on the
    # tensor engine) for ~1.5e-5 output error instead of f32r's ~1e-3.
    use_f32r = os.environ.get("RNN_EXACT", "0") != "1"
    key = ("prog", has_bias, use_f32r)
    if key not in _CACHE:
        _CACHE[key] = _build(has_bias, use_f32r=use_f32r)
    return _CACHE[key]


def _host_xt(shard):
    # shard: [B_LOC, T, F] -> xT [F, TB, NG, GW] with
    # column (t, g, s_loc*NB + blk) = x[g*4 + s_loc, blk*TB + t, :]
    v = shard.reshape(NG, B_LOC // NG, NB, TB, F)
    return np.ascontiguousarray(v.transpose(4, 3, 0, 1, 2).reshape(F, TB, NG, GW))


def kernel(inputs, W, U, b):
    from concourse import bass_utils

    x = np.asarray(inputs, dtype=np.float32)
    W = np.ascontiguousarray(np.asarray(W, dtype=np.float32))
    U = np.ascontiguousarray(np.asarray(U, dtype=np.float32))
    b = np.ascontiguousarray(np.asarray(b, dtype=np.float32))
    assert x.shape == (B_GLOB, T, F), x.shape

    has_bias = bool(np.any(b))
    nc = _get_program(has_bias)

    in_maps = []
    for c in range(NCORES):
        shard = x[c * B_LOC : (c + 1) * B_LOC]
        m = {"xt": _host_xt(shard), "w": W, "u": U}
        if has_bias:
            m["bvec"] = b
        in_maps.append(m)

    res = bass_utils.run_bass_kernel_spmd(nc, in_maps, core_ids=list(range(NCORES)))

    # un-transpose the scan-layout output on the host (shard gather glue):
    # yscr[t, g, u, half*GW + s_loc*NB + blk] -> y[g*4+s_loc, blk*TB+t, half*128+u]
    y = np.empty((B_GLOB, T, H), dtype=np.float32)
    for c in range(NCORES):
        scr = res.results[c]["yscr"]  # [TB, NG, 128, 2*GW]
        scr = scr.reshape(TB, NG, 128, 2, B_LOC // NG, NB)
        # -> [g, s_loc, nb, tb, half, u]
        yc = scr.transpose(1, 4, 5, 0, 3, 2).reshape(B_LOC, T, H)
        y[c * B_LOC : (c + 1) * B_LOC] = yc
    return y


# revision 15
# speedup vs baseline: 1.0068x; 1.0068x over previous
"""Trainium2 Bass kernel for a tanh RNN (h_t = tanh(x_t @ W + h_{t-1} @ U + b)).

Strategy
--------
Data-parallel over batch: 64 sequences -> 8 cores x 8 sequences. W/U/b are
replicated; the recurrent state stays resident per core.

Per core, the T=2048 serial scan is reformulated as a block-Jacobi scan:
split T into NB=64 blocks of TB=32 steps. All blocks are scanned in
parallel (they become extra batch: 8 seqs x 64 blocks = 512 columns per
step), seeded with initial states from the previous sweep (zeros on sweep
0). Two sweeps suffice: the per-step Jacobian ||diag(tanh') U|| ~ 0.7
contracts an O(1) seed error to ~1e-5 across a 32-step block (verified
against the reference scan). Sweep 2 re-scans with block i seeded by
sweep-1's end state of block i-1 and emits the outputs.

Layout: the scan state is kept transposed (hT: units on partitions, batch
in the free dim) so tanh (ScalarE) reads PSUM and writes hT directly with
no per-step transpose. The 512-column batch is split into two independent
groups (s 0-3 / s 4-7) whose chains interleave, so one group's tanh
overlaps the other group's matmuls. Per step and group: 6 matmuls into
one PSUM bank — z.T[uo, col] = W[:, uo].T @ xT_t (start=True injects x@W
and clears the bank) plus four accumulating U-block matmuls — then one
ACT tanh over [128, 512]. Matmuls run as float32r (full-rate fp32; plain
fp32 is 4 cycles/row), which lands ~1e-3 scale-relative error on this
contractive recurrence.

Host-side shard glue: x is uploaded pre-transposed and step-major
(xT[:, t*512 + col]) so the contraction dim lands on partitions, each
step reads a contiguous slab, and the input DMA streams just ahead of
the scan. y is written to DRAM in the scan's native layout and
un-transposed on the host during the gather. All FLOPs stay on device;
the host only does layout.
"""

import os

import numpy as np

B_GLOB = 64
B_LOC = 8
T = 2048
F = 128
H = 256
NCORES = 8
TB = 32                    # block length (chain steps per sweep)
NB = T // TB               # 64 blocks
BATCH = B_LOC * NB         # 512 columns per scan step
NG = 2                     # independent batch groups (interleaved chains)
GW = BATCH // NG           # 256 columns per group
ROWS = B_LOC * T           # 16384 columns of xT per core

_CACHE = {}


def _build(has_bias: bool, use_f32r: bool = True):
    from contextlib import ExitStack

    import concourse.tile as tile
    from concourse import bacc, mybir

    f32 = mybir.dt.float32
    # float32r: same fp32 storage, tensor-engine matmul runs at full rate
    # (plain fp32 is 4 cycles/row) with reduced product precision; fine here
    # because the recurrence is contractive.
    mmdt = mybir.dt.float32r if use_f32r else f32

    def mm(ap):
        return ap if ap.dtype == mmdt else ap.bitcast(mmdt)

    nc = bacc.Bacc(
        "TRN2",
        target_bir_lowering=False,
        debug=False,
        enable_asserts=False,
        num_devices=NCORES,
    )

    # xT columns: t*BATCH + g*GW + s_loc*NB + blk   (step-major, contiguous)
    xT_d = nc.dram_tensor("xt", (F, TB, NG, GW), f32, kind="ExternalInput").ap()
    w_d = nc.dram_tensor("w", (F, H), f32, kind="ExternalInput").ap()
    u_d = nc.dram_tensor("u", (H, H), f32, kind="ExternalInput").ap()
    if has_bias:
        b_d = nc.dram_tensor("bvec", (H,), f32, kind="ExternalInput").ap()
    # y in scan layout: [t][g][u%128][half*GW + s_loc*NB + blk]
    y_d = nc.dram_tensor("yscr", (TB, NG, 128, 2 * GW), f32, kind="ExternalOutput").ap()

    with tile.TileContext(nc) as tc, ExitStack() as ctx:
        consts = ctx.enter_context(tc.tile_pool(name="consts", bufs=1))
        hpool = ctx.enter_context(tc.tile_pool(name="hpool", bufs=6))
        zpsum = ctx.enter_context(tc.tile_pool(name="zpsum", bufs=4, space="PSUM"))

        w_sb = consts.tile([128, H], mmdt)
        nc.sync.dma_start(out=w_sb, in_=mm(w_d))
        u_sb = consts.tile([128, 2, H], mmdt)
        nc.sync.dma_start(out=u_sb, in_=mm(u_d.rearrange("(k p) h -> p k h", p=128)))
        if has_bias:
            b_sb = consts.tile([128, 2], f32)
            nc.sync.dma_start(out=b_sb, in_=b_d.rearrange("(k p) -> p k", p=128))

        # x, pre-transposed/step-major on host: [128(f), TB, NG, GW]
        xT = consts.tile([128, TB, NG, GW], mmdt)
        # stream the load in step-order so the scan can start immediately
        for c in range(TB):
            nc.sync.dma_start(
                out=xT[:, c : c + 1],
                in_=mm(xT_d[:, c : c + 1]),
            )

        tanh = mybir.ActivationFunctionType.Tanh

        zeros_sb = consts.tile([128, 2 * GW], f32)
        nc.vector.memset(zeros_sb, 0.0)

        h_prev = []
        for g in range(NG):
            hp = hpool.tile([128, 2 * GW], mmdt, tag=f"h{g}")
            nc.vector.tensor_copy(out=hp[:], in_=zeros_sb[:])
            h_prev.append(hp)

        for p in range(2):
            final = p == 1
            for t in range(TB):
                for g in range(NG):
                    xmov = xT[:, t, g, :]  # [128, GW] contiguous
                    hp = h_prev[g]
                    z = zpsum.tile([128, 2 * GW], f32, tag=f"z{g}")
                    # inject x_t @ W (start=True clears the bank)
                    nc.tensor.matmul(
                        z[:, 0:GW], lhsT=mm(w_sb[:, 0:128]), rhs=mm(xmov),
                        start=True, stop=False,
                    )
                    nc.tensor.matmul(
                        z[:, GW : 2 * GW], lhsT=mm(w_sb[:, 128:256]), rhs=mm(xmov),
                        start=False, stop=False,
                    )
                    # accumulate h_{t-1} @ U
                    nc.tensor.matmul(
                        z[:, 0:GW], lhsT=mm(u_sb[:, 0, 0:128]),
                        rhs=mm(hp[:, 0:GW]), start=False, stop=False,
                    )
                    nc.tensor.matmul(
                        z[:, 0:GW], lhsT=mm(u_sb[:, 1, 0:128]),
                        rhs=mm(hp[:, GW : 2 * GW]), start=False, stop=False,
                    )
                    nc.tensor.matmul(
                        z[:, GW : 2 * GW], lhsT=mm(u_sb[:, 0, 128:256]),
                        rhs=mm(hp[:, 0:GW]), start=False, stop=False,
                    )
                    nc.tensor.matmul(
                        z[:, GW : 2 * GW], lhsT=mm(u_sb[:, 1, 128:256]),
                        rhs=mm(hp[:, GW : 2 * GW]), start=False, stop=True,
                    )

                    h_cur = hpool.tile([128, 2 * GW], mmdt, tag=f"h{g}")
                    if has_bias:
                        nc.scalar.activation(
                            out=h_cur[:, 0:GW], in_=z[:, 0:GW],
                            func=tanh, bias=b_sb[:, 0:1],
                        )
                        nc.scalar.activation(
                            out=h_cur[:, GW : 2 * GW], in_=z[:, GW : 2 * GW],
                            func=tanh, bias=b_sb[:, 1:2],
                        )
                    else:
                        nc.scalar.activation(out=h_cur[:], in_=z[:], func=tanh)

                    if final:
                        nc.sync.dma_start(
                            out=y_d[t, g],
                            in_=h_cur[:].bitcast(f32),
                        )

                    h_prev[g] = h_cur

            if not final:
                # seed sweep 2: block i <- end state of block i-1; block 0 <- 0
                for g in range(NG):
                    ends = h_prev[g]
                    h_init = hpool.tile([128, 2 * GW], mmdt, tag=f"h{g}")
                    nc.vector.tensor_copy(out=h_init[:], in_=zeros_sb[:])
                    # cols: (half, s_loc, blk); shift blk by one within (half, s_loc)
                    ends_g = ends[:].rearrange("p (q nb) -> p q nb", nb=NB)
                    init_g = h_init[:].rearrange("p (q nb) -> p q nb", nb=NB)
                    nc.vector.tensor_copy(
                        out=init_g[:, :, 1:NB], in_=ends_g[:, :, 0 : NB - 1]
                    )
                    h_prev[g] = h_init

    nc.compile()
    return nc


def _get_program(has_bias: bool):
    # RNN_EXACT=1 switches the matmuls to exact fp32 (4 cycles/row on the
    # tensor engine) for ~1.5e-5 output error instead of f32r's ~1e-3.
    use_f32r = os.environ.get("RNN_EXACT", "0") != "1"
    key = ("prog", has_bias, use_f32r)
    if key not in _CACHE:
        _CACHE[key] = _build(has_bias, use_f32r=use_f32r)
    return _CACHE[key]


def _host_xt(shard):
    # shard: [B_LOC, T, F] -> xT [F, TB, NG, GW] with
    # column (t, g, s_loc*NB + blk) = x[g*4 + s_loc, blk*TB + t, :]
    v = shard.reshape(NG, B_LOC // NG, NB, TB, F)
    return np.ascontiguousarray(v.transpose(4, 3, 0, 1, 2).reshape(F, TB, NG, GW))


def kernel(inputs, W, U, b):
    from concourse import bass_utils

    x = np.asarray(inputs, dtype=np.float32)
    W = np.ascontiguousarray(np.asarray(W, dtype=np.float32))
    U = np.ascontiguousarray(np.asarray(U, dtype=np.float32))
    b = np.ascontiguousarray(np.asarray(b, dtype=np.float32))
    assert x.shape == (B_GLOB, T, F), x.shape

    has_bias = bool(np.any(b))
    nc = _get_program(has_bias)

    in_maps = []
    for c in range(NCORES):
        shard = x[c * B_LOC : (c + 1) * B_LOC]
        m = {"xt": _host_xt(shard), "w": W, "u": U}
        if has_bias:
            m["bvec"] = b
        in_maps.append(m)

    res = bass_utils.run_bass_kernel_spmd(nc, in_maps, core_ids=list(range(NCORES)))

    # un-transpose the scan-layout output on the host (shard gather glue):
    # yscr[t, g, u, half*GW + s_loc*NB + blk] -> y[g*4+s_loc, blk*TB+t, half*128+u]
    y = np.empty((B_GLOB, T, H), dtype=np.float32)
    for c in range(NCORES):
        scr = res.results[c]["yscr"]  # [TB, NG, 128, 2*GW]
        scr = scr.reshape(TB, NG, 128, 2, B_LOC // NG, NB)
        # -> [g, s_loc, nb, tb, half, u]
        yc = scr.transpose(1, 4, 5, 0, 3, 2).reshape(B_LOC, T, H)
        y[c * B_LOC : (c + 1) * B_LOC] = yc
    return y
